# revision 18
# baseline (speedup 1.0000x reference)
"""Trainium2 Bass kernel for AdaptiveWeightedFConLoss (8 NeuronCores, SPMD).

Strategy (sharding_hint): anchor rows are sharded across the 8 cores; each
core owns 768 rows of embF / FM_adj / FP_adj; embM / embP are replicated.
Host-side work is layout-only (dtype cast to fp16, slicing, transposition);
all arithmetic happens on the device.

Per-core device pipeline:
  - l2-norm scales computed on-device (squares -> ones-matmul row sums ->
    exp(-0.5*ln(s)) to stay in one ACT table set).
  - Aggregation (FM_adj @ embM) as PE matmuls over host-transposed adjacency
    tiles; result kept transposed [d, i] which feeds the MLP matmuls directly.
  - Similarity exp(cos/tau): PE matmul -> ScalarE exp with accum_out (row
    totals ride for free) -> fused DVE tensor_tensor_reduce for the masked
    positive sums; degree row-sums via DVE tensor_scalar accum.
  - 2-class softmax via exp+reciprocal (no sigmoid table), loss tail reduced
    on-chip, AllReduce of the scalar partial across the 8 cores.
"""

import numpy as np

N = 6144
D = 128
TAU = 0.1
NCORES = 8
P = 128
R = N // NCORES          # 768 anchor rows per core
GRP = 1024               # sim column group (2 psum banks)

_CACHE = {}


def _patch_tile_drain():
    """walrus in this container only allows one semaphore wait per CTRL
    (Drain) instruction; split the TileContext exit-drain waits across
    single-wait NOPs."""
    from concourse import tile, mybir
    from concourse.tile import ScopedClock

    if getattr(tile.TileContext, "_drain_patched", False):
        return

    def _drain_and_barrier(self, tick_clock, wait_clock):
        nc = self.nc
        drain_inst = nc.sync.drain()
        wait_clock.add_sem_waits(
            drain_inst.ins, ScopedClock({None: tick_clock.global_clock})
        )
        si = drain_inst.ins.sync_info
        if si is not None and si.on_wait is not None and len(si.on_wait) > 1:
            waits = list(si.on_wait)
            del si.on_wait[1:]
            for w in waits[1:]:
                n = nc.sync.nop(nofuse=True)
                n.ins.sync_info = mybir.SyncInfo(on_wait=[w], on_update=[])
        nc.all_engine_barrier()
        popped = nc._tile_sem_poison_stack.pop()
        assert popped is self._sem_poison
        nc.clear_and_free_semaphores(list(self.sems.allocated().values()))
        nc.all_engine_barrier()

    tile.TileContext._drain_and_barrier = _drain_and_barrier
    tile.TileContext._drain_patched = True


def _dedupe_ldweights(nc):
    """Tile legalization inserts an InstLdweights before every matmul, even
    when the PE array already holds those weights; drop the redundant ones so
    same-weight matmuls issue back-to-back."""
    removed = 0
    for fn in nc.m.functions:
        for bb in fn.blocks:
            out = []
            last_key = [None]
            pending = []
            for ins in bb.instructions:
                tn = type(ins).__name__
                if tn == "InstLdweights":
                    ap = ins.ins[0]
                    try:
                        key = (ap.memref, ap.offset, str(ap.ap), str(ap.dtype),
                               ins.is_transpose, ins.perf_mode)
                    except AttributeError:
                        key = object()
                    si = ins.sync_info
                    has_upd = bool(si and si.on_update)
                    if key == last_key[0] and not has_upd:
                        removed += 1
                        if si and si.on_wait:
                            pending.extend(si.on_wait)
                        continue
                    last_key[0] = key
                elif tn == "InstMatmult":
                    if pending:
                        from concourse import mybir
                        si = ins.sync_info
                        if si is None:
                            ins.sync_info = mybir.SyncInfo(
                                on_wait=list(pending), on_update=[])
                        else:
                            si.on_wait = list(si.on_wait or []) + pending
                        pending = []
                out.append(ins)
            assert not pending
            bb.instructions[:] = out
    return removed


def _split_multi_waits(nc, limit=1):
    """This container's walrus allows only one semaphore wait per
    instruction; move extra waits onto same-engine NOPs inserted before."""
    from concourse import mybir

    cnt = 0
    for fn in nc.m.functions:
        for bb in fn.blocks:
            out = []
            for ins in bb.instructions:
                si = ins.sync_info
                if si is not None and si.on_wait and len(si.on_wait) > limit:
                    waits = list(si.on_wait)
                    del si.on_wait[limit:]
                    for w in waits[limit:]:
                        cnt += 1
                        nop = mybir.InstNoOp(
                            name=f"I-wsplit-{cnt}", ins=[], outs=[])
                        nop.engine = ins.engine
                        nop.sync_info = mybir.SyncInfo(
                            on_wait=[w], on_update=[])
                        out.append(nop)
                out.append(ins)
            bb.instructions[:] = out


def _patch_ldw_opt():
    """Enable walrus's LDWEIGHTS dedup pass (hardcoded off in bass_utils):
    consecutive same-weight matmuls then pipeline back-to-back."""
    from concourse import bass_utils

    if getattr(bass_utils, "_ldw_opt_patched", False):
        return
    orig = bass_utils.run_command

    def run_command(cmd, *a, **kw):
        if isinstance(cmd, list):
            cmd = ["--enable-ldw-opt=true" if c == "--enable-ldw-opt=false"
                   else c for c in cmd]
        return orig(cmd, *a, **kw)

    bass_utils.run_command = run_command
    bass_utils._ldw_opt_patched = True


def build(n=N):
    """Build the SPMD Bass program for one core (all cores identical)."""
    from concourse import bass, tile, mybir

    _patch_tile_drain()

    f16 = mybir.dt.float16
    f32 = mybir.dt.float32
    AF = mybir.ActivationFunctionType
    OP = mybir.AluOpType
    AX = mybir.AxisListType.X

    r = n // NCORES
    nblk = r // P
    njc = n // P
    njb = njc // 4
    grp = GRP if n % GRP == 0 else 512
    ngrp = n // grp
    nsub = grp // 512
    inv_tau = 1.0 / TAU

    nc = bass.Bass()
    dp = nc.declare_dram_parameter
    adjF_e = dp("adjF", [r, n], f16, isOutput=False)
    adjP_e = dp("adjP", [r, n], f16, isOutput=False)
    adjFT_e = dp("adjFT", [n // 4, 4 * r], f16, isOutput=False)
    adjPT_e = dp("adjPT", [n // 4, 4 * r], f16, isOutput=False)
    embMn_e = dp("embMn", [P, (n // P) * D], f16, isOutput=False)
    embPn_e = dp("embPn", [P, (n // P) * D], f16, isOutput=False)
    embMT_e = dp("embMT", [P, n], f16, isOutput=False)
    embPT_e = dp("embPT", [P, n], f16, isOutput=False)
    embFT_e = dp("embFT", [P, r], f16, isOutput=False)
    w1m_e = dp("w1m", [P, P], f16, isOutput=False)
    w1p_e = dp("w1p", [P, P], f16, isOutput=False)
    w2_e = dp("w2", [P, 2], f16, isOutput=False)
    b1r_e = dp("b1r", [1, P], f32, isOutput=False)
    b2r_e = dp("b2r", [1, 2], f32, isOutput=False)
    ones16_e = dp("ones16", [P, 1], f16, isOutput=False)
    onescol_e = dp("onescol", [P, 1], f32, isOutput=False)
    ones11_e = dp("ones11", [1, 1], f32, isOutput=False)
    ident_e = dp("ident", [P, P], f16, isOutput=False)
    wout_e = dp("w_out", [r, 2], f32, isOutput=True)
    loss_e = dp("loss_out", [1, 1], f32, isOutput=True)

    with tile.TileContext(nc) as tc:
        with (
            tc.tile_pool(name="const", bufs=1) as cp,
            tc.tile_pool(name="resident", bufs=1) as rp,
            tc.tile_pool(name="stage", bufs=1) as sp,
            tc.tile_pool(name="adjt", bufs=3) as adjtp,
            tc.tile_pool(name="adjb", bufs=4) as adjbp,
            tc.tile_pool(name="sim", bufs=3) as simp,
            tc.tile_pool(name="msk", bufs=2) as mskp,
            tc.tile_pool(name="small", bufs=1) as smp,
            tc.tile_pool(name="dram", bufs=1, space="DRAM") as dramp,
        ):
            # ---- consts ----
            ones16 = cp.tile([P, 1], f16, tag="c0")
            nc.sync.dma_start(out=ones16[:], in_=ones16_e[:])
            onescol = cp.tile([P, 1], f32, tag="c1")
            nc.sync.dma_start(out=onescol[:], in_=onescol_e[:])
            ones11 = cp.tile([1, 1], f32, tag="c2")
            nc.sync.dma_start(out=ones11[:], in_=ones11_e[:])
            ident = cp.tile([P, P], f16, tag="c3")
            nc.sync.dma_start(out=ident[:], in_=ident_e[:])
            w1m = cp.tile([P, P], f16, tag="c4")
            nc.sync.dma_start(out=w1m[:], in_=w1m_e[:])
            w1p = cp.tile([P, P], f16, tag="c5")
            nc.sync.dma_start(out=w1p[:], in_=w1p_e[:])
            w2 = cp.tile([P, 2], f16, tag="c6")
            nc.sync.dma_start(out=w2[:], in_=w2_e[:])
            b2r = cp.tile([1, 2], f32, tag="c8")
            nc.sync.dma_start(out=b2r[:], in_=b2r_e[:])
            b1bc = cp.tile([P, P], f32, tag="c9")
            nc.sync.dma_start(out=b1bc[:], in_=b1r_e[0:1, :].broadcast_to([P, P]))

            # ---- normalization scales via ones-matmul row sums ----
            # rn_row = exp(-0.5*ln(colsum(x^2)))  (stays in the exp/ln table)
            def norm_scale(embT, width, rowps, name):
                sq = sp.tile([P, width], f16, tag="sqx", name="sq")
                nc.vector.tensor_tensor(out=sq[:], in0=embT[:], in1=embT[:],
                                        op=OP.mult)
                rnx = sp.tile([1, width], f16, tag="rnx", name="rnx")
                for s in range(0, width, 512):
                    w = min(512, width - s)
                    nrm = rowps.tile([1, 512], f32, tag="nrm", name="nrm")
                    nc.tensor.matmul(nrm[:, 0:w], lhsT=ones16[:],
                                     rhs=sq[:, s:s + w], start=True, stop=True)
                    lnp = sp.tile([1, 512], f32, tag="lnp", name="lnp")
                    nc.scalar.activation(lnp[0:1, 0:w], nrm[:, 0:w], AF.Ln)
                    nc.scalar.activation(rnx[0:1, s:s + w], lnp[0:1, 0:w],
                                         AF.Exp, scale=-0.5)
                rnd = dramp.tile([1, width], f16, tag="rnd_" + name)
                nc.sync.dma_start(out=rnd[:], in_=rnx[:])
                rnb = sp.tile([P, width], f16, tag="rnb", name="rnb")
                nc.sync.dma_start(out=rnb[:],
                                  in_=rnd[0:1, :].broadcast_to([P, width]))
                out = rp.tile([P, width], f16, tag="nt_" + name)
                nc.vector.tensor_tensor(out=out[:], in0=embT[:], in1=rnb[:],
                                        op=OP.mult)
                return out

            with tc.tile_pool(name="rowps", bufs=2, space="PSUM") as rowps:
                embFT = sp.tile([P, r], f16, tag="eft")
                nc.sync.dma_start(out=embFT[:], in_=embFT_e[:])
                FnT = norm_scale(embFT, r, rowps, "f")
                embMT = sp.tile([P, n], f16, tag="ext", name="embXT")
                nc.sync.dma_start(out=embMT[:], in_=embMT_e[:])
                MnT = norm_scale(embMT, n, rowps, "m")
                embPT = sp.tile([P, n], f16, tag="ext", name="embXT2")
                nc.sync.dma_start(out=embPT[:], in_=embPT_e[:])
                PnT = norm_scale(embPT, n, rowps, "p")

            # ---- interleaved schedule: aggregation batches woven between
            # sim/mask units so PE, ACT, DVE and both DMA streams overlap ----
            rawT = []
            degrows = []
            rw1 = min(512, r)

            with (
                tc.tile_pool(name="aggps", bufs=1, space="PSUM") as aggps,
                tc.tile_pool(name="simps", bufs=2, space="PSUM") as simps,
            ):
                posF_p = smp.tile([P, nblk * ngrp], f32, tag="posf")
                posP_p = smp.tile([P, nblk * ngrp], f32, tag="posp")
                totF_p = smp.tile([P, nblk * ngrp], f32, tag="totf")
                totP_p = smp.tile([P, nblk * ngrp], f32, tag="totp")

                # sim work: per block -> load adj rows, then per (rel, grp)
                sim_state = {"blk": -1, "adjF": None, "adjP": None}
                sim_work = [(blk, ri, g) for blk in range(nblk)
                            for ri in range(2) for g in range(ngrp)]
                sim_pos = [0]

                def emit_block_load(blk):
                    adjFb = adjbp.tile([P, n], f16, tag="adjb", name="adjFb")
                    nc.sync.dma_start(out=adjFb[:],
                                      in_=adjF_e[blk * P:(blk + 1) * P, :])
                    adjPb = adjbp.tile([P, n], f16, tag="adjb", name="adjPb")
                    nc.sync.dma_start(out=adjPb[:],
                                      in_=adjP_e[blk * P:(blk + 1) * P, :])
                    sim_state["blk"] = blk
                    sim_state["adjF"] = adjFb
                    sim_state["adjP"] = adjPb

                def emit_sim(count):
                    for _ in range(count):
                        if sim_pos[0] >= len(sim_work):
                            return
                        blk, ri, g = sim_work[sim_pos[0]]
                        sim_pos[0] += 1
                        if sim_state["blk"] != blk:
                            emit_block_load(blk)
                        adjb = sim_state["adjF"] if ri == 0 else sim_state["adjP"]
                        XT = MnT if ri == 0 else PnT
                        pos_p = posF_p if ri == 0 else posP_p
                        tot_p = totF_p if ri == 0 else totP_p
                        sps = simps.tile([P, grp], f32, tag="simps",
                                         name="sps")
                        for s in range(nsub):
                            c0 = g * grp + s * 512
                            nc.tensor.matmul(
                                sps[:, s * 512:(s + 1) * 512],
                                lhsT=FnT[:, blk * P:(blk + 1) * P],
                                rhs=XT[:, c0:c0 + 512],
                                start=True, stop=True)
                        sim = simp.tile([P, grp], f16, tag="sim", name="sim")
                        idx = blk * ngrp + g
                        nc.scalar.activation(
                            sim[:], sps[:], AF.Exp, scale=inv_tau,
                            accum_out=tot_p[:, idx:idx + 1])
                        msk = mskp.tile([P, grp], f16, tag="msk", name="msk")
                        nc.vector.scalar_tensor_tensor(
                            out=msk[:], in0=sim[:], scalar=1.0,
                            in1=adjb[:, g * grp:(g + 1) * grp],
                            op0=OP.mult, op1=OP.mult,
                            accum_out=pos_p[:, idx:idx + 1])

                # ~3 sim units per agg batch balances PE/ACT/DVE streams
                n_agg_steps = 2 * njb
                per = max(1, (len(sim_work) + n_agg_steps - 1) // n_agg_steps)

                for name, embXn_e, adjXT_e in (("m", embMn_e, adjFT_e),
                                               ("p", embPn_e, adjPT_e)):
                    embXn = rp.tile([P, njc, D], f16, tag="exn_" + name,
                                    name="exn")
                    nc.sync.dma_start(
                        out=embXn[:],
                        in_=embXn_e[:].rearrange("p (a d) -> p a d", d=D))
                    raw1 = aggps.tile([P, rw1], f32, tag="agg1", name="raw1")
                    raw2 = (aggps.tile([P, r - 512], f32, tag="agg2",
                                       name="raw2") if r > 512 else None)
                    degrow = aggps.tile([1, r], f32, tag="degrow",
                                        name="degrow")
                    for jb in range(njb):
                        bt = adjtp.tile([P, 4, r], f16, tag="adjt", name="bt")
                        nc.sync.dma_start(
                            out=bt[:],
                            in_=adjXT_e[jb * P:(jb + 1) * P, :].rearrange(
                                "p (c i) -> p c i", c=4))
                        for c in range(4):
                            jc = jb * 4 + c
                            st = jc == 0
                            en = jc == njc - 1
                            nc.tensor.matmul(degrow[:, 0:rw1], lhsT=ones16[:],
                                             rhs=bt[:, c, 0:rw1],
                                             start=st, stop=en)
                            if r > 512:
                                nc.tensor.matmul(degrow[:, 512:r],
                                                 lhsT=ones16[:],
                                                 rhs=bt[:, c, 512:r],
                                                 start=st, stop=en)
                        for c in range(4):
                            jc = jb * 4 + c
                            st = jc == 0
                            en = jc == njc - 1
                            nc.tensor.matmul(raw1[:], lhsT=embXn[:, jc, :],
                                             rhs=bt[:, c, 0:rw1],
                                             start=st, stop=en)
                            if raw2 is not None:
                                nc.tensor.matmul(raw2[:], lhsT=embXn[:, jc, :],
                                                 rhs=bt[:, c, 512:r],
                                                 start=st, stop=en)
                        emit_sim(per)
                    rt = rp.tile([P, r], f16, tag="rawt_" + name)
                    nc.vector.tensor_copy(rt[:, 0:rw1], raw1[:])
                    if raw2 is not None:
                        nc.vector.tensor_copy(rt[:, 512:r], raw2[:])
                    dr = sp.tile([1, r], f32, tag="degrow_sb_" + name,
                                 name="degrow_sb")
                    nc.vector.tensor_copy(dr[:], degrow[:])
                    rawT.append(rt)
                    degrows.append(dr)
                emit_sim(len(sim_work))

                posF = smp.tile([P, nblk], f32, tag="posfr")
                posP = smp.tile([P, nblk], f32, tag="pospr")
                totF = smp.tile([P, nblk], f32, tag="totfr")
                totP = smp.tile([P, nblk], f32, tag="totpr")
                for dst, src in ((posF, posF_p), (posP, posP_p),
                                 (totF, totF_p), (totP, totP_p)):
                    nc.vector.tensor_reduce(
                        out=dst[:].rearrange("p (b u) -> p b u", u=1),
                        in_=src[:].rearrange("p (b g) -> p b g", g=ngrp),
                        axis=AX, op=OP.add)
            rawTM, rawTP = rawT

            with tc.tile_pool(name="mlpps", bufs=1, space="PSUM") as mlpps:
                # deg rows [1, r] -> column layout [P, nblk] via tiny matmuls
                degF = smp.tile([P, nblk], f32, tag="degf")
                degP = smp.tile([P, nblk], f32, tag="degp")
                for dst, drow in ((degF, degrows[0]), (degP, degrows[1])):
                    dcp = mlpps.tile([P, nblk], f32, tag="degc", name="dcp")
                    for blk in range(nblk):
                        nc.tensor.matmul(dcp[:, blk:blk + 1],
                                         lhsT=drow[0:1, blk * P:(blk + 1) * P],
                                         rhs=ones11[:], start=True, stop=True)
                    nc.vector.tensor_copy(dst[:], dcp[:])

                # ---- MLP -> weights ----
                degFc = smp.tile([P, nblk], f32, tag="degfc")
                nc.vector.tensor_scalar(out=degFc[:], in0=degF[:], scalar1=1.0,
                                        scalar2=None, op0=OP.max)
                degPc = smp.tile([P, nblk], f32, tag="degpc")
                nc.vector.tensor_scalar(out=degPc[:], in0=degP[:], scalar1=1.0,
                                        scalar2=None, op0=OP.max)
                rdF = smp.tile([P, nblk], f32, tag="rdf")
                nc.vector.reciprocal(rdF[:], degFc[:])
                rdP = smp.tile([P, nblk], f32, tag="rdp")
                nc.vector.reciprocal(rdP[:], degPc[:])

                w2d = smp.tile([P, 1], f16, tag="w2d")
                nc.vector.tensor_tensor(out=w2d[:], in0=w2[:, 1:2],
                                        in1=w2[:, 0:1], op=OP.subtract)
                d01r = smp.tile([1, r], f32, tag="d01r")
                for blk in range(nblk):
                    bs = slice(blk * P, (blk + 1) * P)
                    u = mlpps.tile([P, P], f32, tag="mlpuv")
                    nc.tensor.matmul(u[:], lhsT=rawTM[:, bs], rhs=w1m[:],
                                     start=True, stop=True)
                    t1 = smp.tile([P, P], f32, tag="t1")
                    nc.vector.tensor_scalar(out=t1[:], in0=u[:],
                                            scalar1=rdF[:, blk:blk + 1],
                                            scalar2=None, op0=OP.mult)
                    v = mlpps.tile([P, P], f32, tag="mlpuv")
                    nc.tensor.matmul(v[:], lhsT=rawTP[:, bs], rhs=w1p[:],
                                     start=True, stop=True)
                    t2 = smp.tile([P, P], f32, tag="t2")
                    nc.vector.scalar_tensor_tensor(
                        out=t2[:], in0=v[:], scalar=rdP[:, blk:blk + 1],
                        in1=t1[:], op0=OP.mult, op1=OP.add)
                    t3 = smp.tile([P, P], f32, tag="t3")
                    nc.vector.tensor_tensor(out=t3[:], in0=t2[:], in1=b1bc[:],
                                            op=OP.add)
                    h16 = smp.tile([P, P], f16, tag="h16")
                    nc.vector.tensor_scalar(out=h16[:], in0=t3[:], scalar1=0.0,
                                            scalar2=None, op0=OP.max)
                    hT = mlpps.tile([P, P], f16, tag="ht")
                    nc.tensor.transpose(hT[:], h16[:], ident[:])
                    hT16 = smp.tile([P, P], f16, tag="ht16")
                    nc.vector.tensor_copy(hT16[:], hT[:])
                    lgb = mlpps.tile([1, P], f32, tag="lgb")
                    nc.tensor.matmul(lgb[:], lhsT=w2d[:], rhs=hT16[:],
                                     start=True, stop=True)
                    nc.vector.tensor_copy(d01r[0:1, bs], lgb[:])

                # w0 = 1 / (1 + exp((l1+b2[1]) - (l0+b2[0])))
                d01 = smp.tile([1, r], f32, tag="d01")
                nc.vector.tensor_scalar(out=d01[:], in0=d01r[:],
                                        scalar1=b2r[0:1, 1:2],
                                        scalar2=b2r[0:1, 0:1],
                                        op0=OP.add, op1=OP.subtract)
                dps = mlpps.tile([P, nblk], f32, tag="dps")
                for blk in range(nblk):
                    nc.tensor.matmul(dps[:, blk:blk + 1],
                                     lhsT=d01[0:1, blk * P:(blk + 1) * P],
                                     rhs=ones11[:], start=True, stop=True)
                eD = smp.tile([P, nblk], f32, tag="ed")
                nc.scalar.activation(eD[:], dps[:], AF.Exp)
                den = smp.tile([P, nblk], f32, tag="den")
                nc.vector.tensor_scalar(out=den[:], in0=eD[:], scalar1=1.0,
                                        scalar2=None, op0=OP.add)
                w0 = smp.tile([P, nblk], f32, tag="w0")
                nc.vector.reciprocal(w0[:], den[:])
                w1c = smp.tile([P, nblk], f32, tag="w1c")
                nc.vector.tensor_scalar(out=w1c[:], in0=w0[:], scalar1=-1.0,
                                        scalar2=1.0, op0=OP.mult, op1=OP.add)

                wout = smp.tile([P, nblk, 2], f32, tag="wout")
                nc.vector.tensor_copy(wout[:, :, 0:1],
                                      w0[:].rearrange("p (b u) -> p b u", u=1))
                nc.vector.tensor_copy(wout[:, :, 1:2],
                                      w1c[:].rearrange("p (b u) -> p b u", u=1))
                nc.sync.dma_start(
                    out=wout_e[:].rearrange("(b p) c -> p b c", p=P),
                    in_=wout[:])

                # ---- loss tail ----
                wp = smp.tile([P, nblk], f32, tag="wp")
                t4 = smp.tile([P, nblk], f32, tag="t4")
                nc.vector.tensor_tensor(out=wp[:], in0=w0[:], in1=posF[:],
                                        op=OP.mult)
                nc.vector.tensor_tensor(out=t4[:], in0=w1c[:], in1=posP[:],
                                        op=OP.mult)
                nc.vector.tensor_tensor(out=wp[:], in0=wp[:], in1=t4[:],
                                        op=OP.add)
                dn = smp.tile([P, nblk], f32, tag="dn")
                nc.vector.tensor_tensor(out=dn[:], in0=w0[:], in1=totF[:],
                                        op=OP.mult)
                nc.vector.tensor_tensor(out=t4[:], in0=w1c[:], in1=totP[:],
                                        op=OP.mult)
                nc.vector.tensor_tensor(out=dn[:], in0=dn[:], in1=t4[:],
                                        op=OP.add)
                nei = smp.tile([P, nblk], f32, tag="nei")
                nc.vector.tensor_tensor(out=nei[:], in0=degF[:], in1=degP[:],
                                        op=OP.add)
                nc.vector.tensor_scalar(out=nei[:], in0=nei[:], scalar1=1.0,
                                        scalar2=None, op0=OP.max)
                nc.vector.tensor_tensor(out=dn[:], in0=dn[:], in1=nei[:],
                                        op=OP.mult)
                rec = smp.tile([P, nblk], f32, tag="rec")
                nc.vector.reciprocal(rec[:], dn[:])
                ratio = smp.tile([P, nblk], f32, tag="ratio")
                nc.vector.tensor_tensor(out=ratio[:], in0=wp[:], in1=rec[:],
                                        op=OP.mult)
                nc.vector.tensor_scalar(out=ratio[:], in0=ratio[:],
                                        scalar1=1e-10, scalar2=None, op0=OP.max)
                lnr = smp.tile([P, nblk], f32, tag="lnr")
                lsum = smp.tile([P, 1], f32, tag="lsum")
                nc.scalar.activation(lnr[:], ratio[:], AF.Ln, accum_out=lsum[:])
                lp = mlpps.tile([1, 1], f32, tag="dps")
                nc.tensor.matmul(lp[:], lhsT=lsum[:], rhs=onescol[:],
                                 start=True, stop=True)
                lps = smp.tile([1, 1], f32, tag="lps")
                nc.vector.tensor_scalar(out=lps[:], in0=lp[:],
                                        scalar1=-1.0 / n, scalar2=None,
                                        op0=OP.mult)

                ib = dramp.tile([1, 1], f32, tag="ib")
                ob = dramp.tile([1, 1], f32, tag="ob")
                nc.gpsimd.dma_start(out=ib[:], in_=lps[:])
                nc.gpsimd.collective_compute(
                    "AllReduce", OP.add,
                    replica_groups=[list(range(NCORES))],
                    ins=[ib.opt()], outs=[ob.opt()])
                nc.gpsimd.dma_start(out=loss_e[:], in_=ob[:])

    _dedupe_ldweights(nc)
    _split_multi_waits(nc)
    return nc


def _prep_inputs(embF, embM, embP, FM_adj, FP_adj, W1, b1, W2, b2, n=N):
    """Host-side layout-only prep: dtype casts, slices, transposes."""
    r = n // NCORES
    f16 = np.float16
    f32 = np.float32
    aF = FM_adj.astype(f16)
    aP = FP_adj.astype(f16)
    def swz_emb(e):
        # [n, D] -> [128, (n/128)*D]: row p holds rows {a*128+p} concatenated
        x = e.astype(f16).reshape(n // P, P, D).transpose(1, 0, 2)
        return np.ascontiguousarray(x.reshape(P, (n // P) * D))

    def swz_adjT(aT):
        # [n, r] -> [n/4, 4r]: row (jb*128+p) holds chunks c=0..3 of superblock
        x = aT.reshape(n // 512, 4, P, r).transpose(0, 2, 1, 3)
        return np.ascontiguousarray(x.reshape(n // 4, 4 * r))

    shared = {
        "embMn": swz_emb(embM),
        "embPn": swz_emb(embP),
        "embMT": np.ascontiguousarray(embM.T.astype(f16)),
        "embPT": np.ascontiguousarray(embP.T.astype(f16)),
        "w1m": np.ascontiguousarray(W1[:D].astype(f16)),
        "w1p": np.ascontiguousarray(W1[D:].astype(f16)),
        "w2": np.ascontiguousarray(W2.astype(f16)),
        "b1r": np.ascontiguousarray(b1.reshape(1, D).astype(f32)),
        "b2r": np.ascontiguousarray(b2.reshape(1, 2).astype(f32)),
        "ones16": np.ones((P, 1), f16),
        "onescol": np.ones((P, 1), f32),
        "ones11": np.ones((1, 1), f32),
        "ident": np.eye(P, dtype=f16),
    }
    in_maps = []
    for c in range(NCORES):
        I = slice(c * r, (c + 1) * r)
        m = dict(shared)
        m["adjF"] = np.ascontiguousarray(aF[I])
        m["adjP"] = np.ascontiguousarray(aP[I])
        m["adjFT"] = swz_adjT(np.ascontiguousarray(aF[I].T))
        m["adjPT"] = swz_adjT(np.ascontiguousarray(aP[I].T))
        m["embFT"] = np.ascontiguousarray(embF[I].T.astype(f16))
        in_maps.append(m)
    return in_maps


def run(inputs, n=N, trace=False, trace_kwargs=None):
    from concourse.bass_utils import run_bass_kernel_spmd

    key = n
    if key not in _CACHE:
        _CACHE[key] = build(n)
    nc = _CACHE[key]
    in_maps = _prep_inputs(n=n, **inputs)
    res = run_bass_kernel_spmd(nc, in_maps, list(range(NCORES)),
                               trace=trace, **(trace_kwargs or {}))
    r = n // NCORES
    weights = np.concatenate(
        [res.results[c]["w_out"] for c in range(NCORES)], axis=0)
    loss = np.float32(res.results[0]["loss_out"][0, 0])
    return loss, weights.astype(np.float32), res


def kernel(embF, embM, embP, FM_adj, FP_adj, W1, b1, W2, b2):
    loss, weights, _ = run(dict(
        embF=np.asarray(embF), embM=np.asarray(embM), embP=np.asarray(embP),
        FM_adj=np.asarray(FM_adj), FP_adj=np.asarray(FP_adj),
        W1=np.asarray(W1), b1=np.asarray(b1), W2=np.asarray(W2),
        b2=np.asarray(b2)))
    return loss, weights


# revision 21
# speedup vs baseline: 1.4372x; 1.4372x over previous
"""Trainium2 Bass kernel for AdaptiveWeightedFConLoss (8 NeuronCores, SPMD).

Strategy (sharding_hint): anchor rows are sharded across the 8 cores; each
core owns 768 rows of embF / FM_adj / FP_adj; embM / embP are replicated.
Host-side work is layout-only (dtype cast to fp16, slicing, transposition);
all arithmetic happens on the device.

Per-core device pipeline:
  - l2-norm scales computed on-device (squares -> ones-matmul row sums ->
    exp(-0.5*ln(s)) to stay in one ACT table set).
  - Aggregation (FM_adj @ embM) as PE matmuls over host-transposed adjacency
    tiles; result kept transposed [d, i] which feeds the MLP matmuls directly.
  - Similarity exp(cos/tau): PE matmul -> ScalarE exp with accum_out (row
    totals ride for free) -> fused DVE tensor_tensor_reduce for the masked
    positive sums; degree row-sums via DVE tensor_scalar accum.
  - 2-class softmax via exp+reciprocal (no sigmoid table), loss tail reduced
    on-chip, AllReduce of the scalar partial across the 8 cores.
"""

import numpy as np

N = 6144
D = 128
TAU = 0.1
NCORES = 8
P = 128
R = N // NCORES          # 768 anchor rows per core
GRP = 1024               # sim column group (2 psum banks)

_CACHE = {}


def _patch_tile_drain():
    """walrus in this container only allows one semaphore wait per CTRL
    (Drain) instruction; split the TileContext exit-drain waits across
    single-wait NOPs."""
    from concourse import tile, mybir
    from concourse.tile import ScopedClock

    if getattr(tile.TileContext, "_drain_patched", False):
        return

    def _drain_and_barrier(self, tick_clock, wait_clock):
        nc = self.nc
        drain_inst = nc.sync.drain()
        wait_clock.add_sem_waits(
            drain_inst.ins, ScopedClock({None: tick_clock.global_clock})
        )
        si = drain_inst.ins.sync_info
        if si is not None and si.on_wait is not None and len(si.on_wait) > 1:
            waits = list(si.on_wait)
            del si.on_wait[1:]
            for w in waits[1:]:
                n = nc.sync.nop(nofuse=True)
                n.ins.sync_info = mybir.SyncInfo(on_wait=[w], on_update=[])
        nc.all_engine_barrier()
        popped = nc._tile_sem_poison_stack.pop()
        assert popped is self._sem_poison
        nc.clear_and_free_semaphores(list(self.sems.allocated().values()))
        nc.all_engine_barrier()

    tile.TileContext._drain_and_barrier = _drain_and_barrier
    tile.TileContext._drain_patched = True


def _dedupe_ldweights(nc):
    """Tile legalization inserts an InstLdweights before every matmul, even
    when the PE array already holds those weights; drop the redundant ones so
    same-weight matmuls issue back-to-back."""
    removed = 0
    for fn in nc.m.functions:
        for bb in fn.blocks:
            out = []
            last_key = [None]
            pending = []
            for ins in bb.instructions:
                tn = type(ins).__name__
                if tn == "InstLdweights":
                    ap = ins.ins[0]
                    try:
                        key = (ap.memref, ap.offset, str(ap.ap), str(ap.dtype),
                               ins.is_transpose, ins.perf_mode)
                    except AttributeError:
                        key = object()
                    si = ins.sync_info
                    has_upd = bool(si and si.on_update)
                    if key == last_key[0] and not has_upd:
                        removed += 1
                        if si and si.on_wait:
                            pending.extend(si.on_wait)
                        continue
                    last_key[0] = key
                elif tn == "InstMatmult":
                    if pending:
                        from concourse import mybir
                        si = ins.sync_info
                        if si is None:
                            ins.sync_info = mybir.SyncInfo(
                                on_wait=list(pending), on_update=[])
                        else:
                            si.on_wait = list(si.on_wait or []) + pending
                        pending = []
                out.append(ins)
            assert not pending
            bb.instructions[:] = out
    return removed


def _merge_mm_incs(nc):
    """Every matmul gets a completion-semaphore update from Tile, which
    forces isolated-instruction latency ((398+N)/2.4 instead of N/2.4).
    Within a run of consecutive PE matmuls/ldweights with no intervening
    semaphore waits, defer all updates onto the last matmul of the run —
    matmuls complete in order, so this is equivalent and lets earlier
    ones pipeline."""
    from concourse import mybir

    for fn in nc.m.functions:
        for bb in fn.blocks:
            pe = [ins for ins in bb.instructions
                  if ins.engine == mybir.EngineType.PE]
            carry = []
            prev_mm = None
            for ins in pe:
                tn = type(ins).__name__
                si = ins.sync_info
                has_wait = bool(si and si.on_wait)
                if tn == "InstMatmult" and not has_wait and prev_mm is not None:
                    psi = prev_mm.sync_info
                    if psi is not None and psi.on_update:
                        carry.extend(psi.on_update)
                        del psi.on_update[:]
                    prev_mm = ins
                elif tn == "InstMatmult":
                    # wait present: flush carry onto the previous matmul
                    if carry and prev_mm is not None:
                        psi = prev_mm.sync_info
                        if psi is None:
                            prev_mm.sync_info = mybir.SyncInfo(
                                on_wait=[], on_update=list(carry))
                        else:
                            psi.on_update = list(psi.on_update or []) + carry
                        carry = []
                    prev_mm = ins
                elif tn in ("InstLdweights", "InstNoOp") and not has_wait:
                    continue
                else:
                    # anything else (or waiting inst) ends the run
                    if carry and prev_mm is not None:
                        psi = prev_mm.sync_info
                        if psi is None:
                            prev_mm.sync_info = mybir.SyncInfo(
                                on_wait=[], on_update=list(carry))
                        else:
                            psi.on_update = list(psi.on_update or []) + carry
                        carry = []
                    prev_mm = None
            if carry and prev_mm is not None:
                psi = prev_mm.sync_info
                if psi is None:
                    prev_mm.sync_info = mybir.SyncInfo(
                        on_wait=[], on_update=list(carry))
                else:
                    psi.on_update = list(psi.on_update or []) + carry
            # fuse same-semaphore updates (HW allows one update per inst)
            for ins in pe:
                si = ins.sync_info
                if si is None or not si.on_update or len(si.on_update) < 2:
                    continue
                fused = {}
                order = []
                for u in si.on_update:
                    k = (u.sync_type, u.id, u.update_mode)
                    if k in fused:
                        fused[k].update_value += u.update_value
                    else:
                        fused[k] = u
                        order.append(k)
                si.on_update[:] = [fused[k] for k in order]


def _split_multi_waits(nc, limit=1):
    """This container's walrus allows only one semaphore wait per
    instruction; move extra waits onto same-engine NOPs inserted before."""
    from concourse import mybir

    cnt = 0
    for fn in nc.m.functions:
        for bb in fn.blocks:
            out = []
            for ins in bb.instructions:
                si = ins.sync_info
                if si is not None and si.on_wait and len(si.on_wait) > limit:
                    waits = list(si.on_wait)
                    del si.on_wait[limit:]
                    for w in waits[limit:]:
                        cnt += 1
                        nop = mybir.InstNoOp(
                            name=f"I-wsplit-{cnt}", ins=[], outs=[])
                        nop.engine = ins.engine
                        nop.sync_info = mybir.SyncInfo(
                            on_wait=[w], on_update=[])
                        out.append(nop)
                out.append(ins)
            bb.instructions[:] = out


def _patch_ldw_opt():
    """Enable walrus's LDWEIGHTS dedup pass (hardcoded off in bass_utils):
    consecutive same-weight matmuls then pipeline back-to-back."""
    from concourse import bass_utils

    if getattr(bass_utils, "_ldw_opt_patched", False):
        return
    orig = bass_utils.run_command

    def run_command(cmd, *a, **kw):
        if isinstance(cmd, list):
            cmd = ["--enable-ldw-opt=true" if c == "--enable-ldw-opt=false"
                   else c for c in cmd]
        return orig(cmd, *a, **kw)

    bass_utils.run_command = run_command
    bass_utils._ldw_opt_patched = True


def build(n=N):
    """Build the SPMD Bass program for one core (all cores identical)."""
    from concourse import bass, tile, mybir

    _patch_tile_drain()

    f16 = mybir.dt.float16
    f32 = mybir.dt.float32
    AF = mybir.ActivationFunctionType
    OP = mybir.AluOpType
    AX = mybir.AxisListType.X

    r = n // NCORES
    nblk = r // P
    njc = n // P
    njb = njc // 4
    grp = GRP if n % GRP == 0 else 512
    ngrp = n // grp
    nsub = grp // 512
    inv_tau = 1.0 / TAU

    nc = bass.Bass()
    dp = nc.declare_dram_parameter
    adjF_e = dp("adjF", [r, n], f16, isOutput=False)
    adjP_e = dp("adjP", [r, n], f16, isOutput=False)
    adjFT_e = dp("adjFT", [n // 4, 4 * r], f16, isOutput=False)
    adjPT_e = dp("adjPT", [n // 4, 4 * r], f16, isOutput=False)
    embMn_e = dp("embMn", [P, (n // P) * D], f16, isOutput=False)
    embPn_e = dp("embPn", [P, (n // P) * D], f16, isOutput=False)
    embMT_e = dp("embMT", [P, n], f16, isOutput=False)
    embPT_e = dp("embPT", [P, n], f16, isOutput=False)
    embFT_e = dp("embFT", [P, r], f16, isOutput=False)
    w1m_e = dp("w1m", [P, P], f16, isOutput=False)
    w1p_e = dp("w1p", [P, P], f16, isOutput=False)
    w2_e = dp("w2", [P, 2], f16, isOutput=False)
    b1r_e = dp("b1r", [1, P], f32, isOutput=False)
    b2r_e = dp("b2r", [1, 2], f32, isOutput=False)
    ones16_e = dp("ones16", [P, 1], f16, isOutput=False)
    onescol_e = dp("onescol", [P, 1], f32, isOutput=False)
    ones11_e = dp("ones11", [1, 1], f32, isOutput=False)
    ident_e = dp("ident", [P, P], f16, isOutput=False)
    wout_e = dp("w_out", [r, 2], f32, isOutput=True)
    loss_e = dp("loss_out", [1, 1], f32, isOutput=True)

    with tile.TileContext(nc) as tc:
        with (
            tc.tile_pool(name="const", bufs=1) as cp,
            tc.tile_pool(name="resident", bufs=1) as rp,
            tc.tile_pool(name="stage", bufs=1) as sp,
            tc.tile_pool(name="adjt", bufs=3) as adjtp,
            tc.tile_pool(name="adjb", bufs=4) as adjbp,
            tc.tile_pool(name="sim", bufs=3) as simp,
            tc.tile_pool(name="msk", bufs=2) as mskp,
            tc.tile_pool(name="small", bufs=1) as smp,
            tc.tile_pool(name="dram", bufs=1, space="DRAM") as dramp,
        ):
            # ---- consts ----
            ones16 = cp.tile([P, 1], f16, tag="c0")
            nc.sync.dma_start(out=ones16[:], in_=ones16_e[:])
            onescol = cp.tile([P, 1], f32, tag="c1")
            nc.sync.dma_start(out=onescol[:], in_=onescol_e[:])
            ones11 = cp.tile([1, 1], f32, tag="c2")
            nc.sync.dma_start(out=ones11[:], in_=ones11_e[:])
            ident = cp.tile([P, P], f16, tag="c3")
            nc.sync.dma_start(out=ident[:], in_=ident_e[:])
            w1m = cp.tile([P, P], f16, tag="c4")
            nc.sync.dma_start(out=w1m[:], in_=w1m_e[:])
            w1p = cp.tile([P, P], f16, tag="c5")
            nc.sync.dma_start(out=w1p[:], in_=w1p_e[:])
            w2 = cp.tile([P, 2], f16, tag="c6")
            nc.sync.dma_start(out=w2[:], in_=w2_e[:])
            b2r = cp.tile([1, 2], f32, tag="c8")
            nc.sync.dma_start(out=b2r[:], in_=b2r_e[:])
            b1bc = cp.tile([P, P], f32, tag="c9")
            nc.sync.dma_start(out=b1bc[:], in_=b1r_e[0:1, :].broadcast_to([P, P]))

            # warm-up AllReduce: establishes the collective channel and
            # absorbs cross-core launch skew while compute streams
            wu = smp.tile([1, 1], f32, tag="wu")
            nc.vector.memset(wu[:], 0.0)
            wub_i = dramp.tile([1, 1], f32, tag="wub_i")
            wub_o = dramp.tile([1, 1], f32, tag="wub_o")
            nc.gpsimd.dma_start(out=wub_i[:], in_=wu[:])
            nc.gpsimd.collective_compute(
                "AllReduce", OP.add,
                replica_groups=[list(range(NCORES))],
                ins=[wub_i.opt()], outs=[wub_o.opt()])

            # ---- normalization scales via ones-matmul row sums ----
            # rn_row = exp(-0.5*ln(colsum(x^2)))  (stays in the exp/ln table)
            def norm_scale(embT, width, rowps, name):
                sq = sp.tile([P, width], f16, tag="sqx", name="sq")
                nc.vector.tensor_tensor(out=sq[:], in0=embT[:], in1=embT[:],
                                        op=OP.mult)
                rnx = sp.tile([1, width], f16, tag="rnx", name="rnx")
                for s in range(0, width, 512):
                    w = min(512, width - s)
                    nrm = rowps.tile([1, 512], f32, tag="nrm", name="nrm")
                    nc.tensor.matmul(nrm[:, 0:w], lhsT=ones16[:],
                                     rhs=sq[:, s:s + w], start=True, stop=True)
                    lnp = sp.tile([1, 512], f32, tag="lnp", name="lnp")
                    nc.scalar.activation(lnp[0:1, 0:w], nrm[:, 0:w], AF.Ln)
                    nc.scalar.activation(rnx[0:1, s:s + w], lnp[0:1, 0:w],
                                         AF.Exp, scale=-0.5)
                rnd = dramp.tile([1, width], f16, tag="rnd_" + name)
                nc.sync.dma_start(out=rnd[:], in_=rnx[:])
                rnb = sp.tile([P, width], f16, tag="rnb", name="rnb")
                nc.sync.dma_start(out=rnb[:],
                                  in_=rnd[0:1, :].broadcast_to([P, width]))
                out = rp.tile([P, width], f16, tag="nt_" + name)
                nc.vector.tensor_tensor(out=out[:], in0=embT[:], in1=rnb[:],
                                        op=OP.mult)
                return out

            with tc.tile_pool(name="rowps", bufs=2, space="PSUM") as rowps:
                embFT = sp.tile([P, r], f16, tag="eft")
                nc.sync.dma_start(out=embFT[:], in_=embFT_e[:])
                FnT = norm_scale(embFT, r, rowps, "f")
                embMT = sp.tile([P, n], f16, tag="ext", name="embXT")
                nc.sync.dma_start(out=embMT[:], in_=embMT_e[:])
                MnT = norm_scale(embMT, n, rowps, "m")
                embPT = sp.tile([P, n], f16, tag="ext", name="embXT2")
                nc.sync.dma_start(out=embPT[:], in_=embPT_e[:])
                PnT = norm_scale(embPT, n, rowps, "p")

            # ---- interleaved schedule: aggregation batches woven between
            # sim/mask units so PE, ACT, DVE and both DMA streams overlap ----
            rawT = []
            degrows = []
            rw1 = min(512, r)

            with (
                tc.tile_pool(name="aggps", bufs=1, space="PSUM") as aggps,
                tc.tile_pool(name="simps", bufs=2, space="PSUM") as simps,
            ):
                posF_p = smp.tile([P, nblk * ngrp], f32, tag="posf")
                posP_p = smp.tile([P, nblk * ngrp], f32, tag="posp")
                totF_p = smp.tile([P, nblk * ngrp], f32, tag="totf")
                totP_p = smp.tile([P, nblk * ngrp], f32, tag="totp")

                # sim work: per block -> load adj rows, then per (rel, grp)
                sim_state = {"blk": -1, "adjF": None, "adjP": None}
                sim_work = [(blk, ri, g) for blk in range(nblk)
                            for ri in range(2) for g in range(ngrp)]
                sim_pos = [0]

                def emit_block_load(blk):
                    adjFb = adjbp.tile([P, n], f16, tag="adjb", name="adjFb")
                    nc.sync.dma_start(out=adjFb[:],
                                      in_=adjF_e[blk * P:(blk + 1) * P, :])
                    adjPb = adjbp.tile([P, n], f16, tag="adjb", name="adjPb")
                    nc.sync.dma_start(out=adjPb[:],
                                      in_=adjP_e[blk * P:(blk + 1) * P, :])
                    sim_state["blk"] = blk
                    sim_state["adjF"] = adjFb
                    sim_state["adjP"] = adjPb

                def emit_sim(count):
                    for _ in range(count):
                        if sim_pos[0] >= len(sim_work):
                            return
                        blk, ri, g = sim_work[sim_pos[0]]
                        sim_pos[0] += 1
                        if sim_state["blk"] != blk:
                            emit_block_load(blk)
                        adjb = sim_state["adjF"] if ri == 0 else sim_state["adjP"]
                        XT = MnT if ri == 0 else PnT
                        pos_p = posF_p if ri == 0 else posP_p
                        tot_p = totF_p if ri == 0 else totP_p
                        sps = simps.tile([P, grp], f32, tag="simps",
                                         name="sps")
                        for s in range(nsub):
                            c0 = g * grp + s * 512
                            nc.tensor.matmul(
                                sps[:, s * 512:(s + 1) * 512],
                                lhsT=FnT[:, blk * P:(blk + 1) * P],
                                rhs=XT[:, c0:c0 + 512],
                                start=True, stop=True)
                        sim = simp.tile([P, grp], f16, tag="sim", name="sim")
                        idx = blk * ngrp + g
                        nc.scalar.activation(
                            sim[:], sps[:], AF.Exp, scale=inv_tau,
                            accum_out=tot_p[:, idx:idx + 1])
                        msk = mskp.tile([P, grp], f16, tag="msk", name="msk")
                        nc.vector.scalar_tensor_tensor(
                            out=msk[:], in0=sim[:], scalar=1.0,
                            in1=adjb[:, g * grp:(g + 1) * grp],
                            op0=OP.mult, op1=OP.mult,
                            accum_out=pos_p[:, idx:idx + 1])

                # ~3 sim units per agg batch balances PE/ACT/DVE streams
                n_agg_steps = 2 * njb
                per = max(1, (len(sim_work) + n_agg_steps - 1) // n_agg_steps)

                for name, embXn_e, adjXT_e in (("m", embMn_e, adjFT_e),
                                               ("p", embPn_e, adjPT_e)):
                    embXn = rp.tile([P, njc, D], f16, tag="exn_" + name,
                                    name="exn")
                    nc.sync.dma_start(
                        out=embXn[:],
                        in_=embXn_e[:].rearrange("p (a d) -> p a d", d=D))
                    raw1 = aggps.tile([P, rw1], f32, tag="agg1", name="raw1")
                    raw2 = (aggps.tile([P, r - 512], f32, tag="agg2",
                                       name="raw2") if r > 512 else None)
                    degrow = aggps.tile([1, r], f32, tag="degrow",
                                        name="degrow")
                    for jb in range(njb):
                        bt = adjtp.tile([P, 4, r], f16, tag="adjt", name="bt")
                        nc.sync.dma_start(
                            out=bt[:],
                            in_=adjXT_e[jb * P:(jb + 1) * P, :].rearrange(
                                "p (c i) -> p c i", c=4))
                        for c in range(4):
                            jc = jb * 4 + c
                            st = jc == 0
                            en = jc == njc - 1
                            nc.tensor.matmul(degrow[:, 0:rw1], lhsT=ones16[:],
                                             rhs=bt[:, c, 0:rw1],
                                             start=st, stop=en)
                            if r > 512:
                                nc.tensor.matmul(degrow[:, 512:r],
                                                 lhsT=ones16[:],
                                                 rhs=bt[:, c, 512:r],
                                                 start=st, stop=en)
                        for c in range(4):
                            jc = jb * 4 + c
                            st = jc == 0
                            en = jc == njc - 1
                            nc.tensor.matmul(raw1[:], lhsT=embXn[:, jc, :],
                                             rhs=bt[:, c, 0:rw1],
                                             start=st, stop=en)
                            if raw2 is not None:
                                nc.tensor.matmul(raw2[:], lhsT=embXn[:, jc, :],
                                                 rhs=bt[:, c, 512:r],
                                                 start=st, stop=en)
                        emit_sim(per)
                    rt = rp.tile([P, r], f16, tag="rawt_" + name)
                    nc.vector.tensor_copy(rt[:, 0:rw1], raw1[:])
                    if raw2 is not None:
                        nc.vector.tensor_copy(rt[:, 512:r], raw2[:])
                    dr = sp.tile([1, r], f32, tag="degrow_sb_" + name,
                                 name="degrow_sb")
                    nc.vector.tensor_copy(dr[:], degrow[:])
                    rawT.append(rt)
                    degrows.append(dr)
                emit_sim(len(sim_work))

                posF = smp.tile([P, nblk], f32, tag="posfr")
                posP = smp.tile([P, nblk], f32, tag="pospr")
                totF = smp.tile([P, nblk], f32, tag="totfr")
                totP = smp.tile([P, nblk], f32, tag="totpr")
                for dst, src in ((posF, posF_p), (posP, posP_p),
                                 (totF, totF_p), (totP, totP_p)):
                    nc.vector.tensor_reduce(
                        out=dst[:].rearrange("p (b u) -> p b u", u=1),
                        in_=src[:].rearrange("p (b g) -> p b g", g=ngrp),
                        axis=AX, op=OP.add)
            rawTM, rawTP = rawT

            with tc.tile_pool(name="mlpps", bufs=1, space="PSUM") as mlpps:
                # deg rows [1, r] -> column layout [P, nblk] via tiny matmuls
                degF = smp.tile([P, nblk], f32, tag="degf")
                degP = smp.tile([P, nblk], f32, tag="degp")
                for dst, drow in ((degF, degrows[0]), (degP, degrows[1])):
                    dcp = mlpps.tile([P, nblk], f32, tag="degc", name="dcp")
                    for blk in range(nblk):
                        nc.tensor.matmul(dcp[:, blk:blk + 1],
                                         lhsT=drow[0:1, blk * P:(blk + 1) * P],
                                         rhs=ones11[:], start=True, stop=True)
                    nc.vector.tensor_copy(dst[:], dcp[:])

                # ---- MLP -> weights ----
                degFc = smp.tile([P, nblk], f32, tag="degfc")
                nc.vector.tensor_scalar(out=degFc[:], in0=degF[:], scalar1=1.0,
                                        scalar2=None, op0=OP.max)
                degPc = smp.tile([P, nblk], f32, tag="degpc")
                nc.vector.tensor_scalar(out=degPc[:], in0=degP[:], scalar1=1.0,
                                        scalar2=None, op0=OP.max)
                rdF = smp.tile([P, nblk], f32, tag="rdf")
                nc.vector.reciprocal(rdF[:], degFc[:])
                rdP = smp.tile([P, nblk], f32, tag="rdp")
                nc.vector.reciprocal(rdP[:], degPc[:])

                w2d = smp.tile([P, 1], f16, tag="w2d")
                nc.vector.tensor_tensor(out=w2d[:], in0=w2[:, 1:2],
                                        in1=w2[:, 0:1], op=OP.subtract)
                d01r = smp.tile([1, r], f32, tag="d01r")
                for blk in range(nblk):
                    bs = slice(blk * P, (blk + 1) * P)
                    u = mlpps.tile([P, P], f32, tag="mlpuv")
                    nc.tensor.matmul(u[:], lhsT=rawTM[:, bs], rhs=w1m[:],
                                     start=True, stop=True)
                    t1 = smp.tile([P, P], f32, tag="t1")
                    nc.vector.tensor_scalar(out=t1[:], in0=u[:],
                                            scalar1=rdF[:, blk:blk + 1],
                                            scalar2=None, op0=OP.mult)
                    v = mlpps.tile([P, P], f32, tag="mlpuv")
                    nc.tensor.matmul(v[:], lhsT=rawTP[:, bs], rhs=w1p[:],
                                     start=True, stop=True)
                    t2 = smp.tile([P, P], f32, tag="t2")
                    nc.vector.scalar_tensor_tensor(
                        out=t2[:], in0=v[:], scalar=rdP[:, blk:blk + 1],
                        in1=t1[:], op0=OP.mult, op1=OP.add)
                    t3 = smp.tile([P, P], f32, tag="t3")
                    nc.vector.tensor_tensor(out=t3[:], in0=t2[:], in1=b1bc[:],
                                            op=OP.add)
                    h16 = smp.tile([P, P], f16, tag="h16")
                    nc.vector.tensor_scalar(out=h16[:], in0=t3[:], scalar1=0.0,
                                            scalar2=None, op0=OP.max)
                    hT = mlpps.tile([P, P], f16, tag="ht")
                    nc.tensor.transpose(hT[:], h16[:], ident[:])
                    hT16 = smp.tile([P, P], f16, tag="ht16")
                    nc.vector.tensor_copy(hT16[:], hT[:])
                    lgb = mlpps.tile([1, P], f32, tag="lgb")
                    nc.tensor.matmul(lgb[:], lhsT=w2d[:], rhs=hT16[:],
                                     start=True, stop=True)
                    nc.vector.tensor_copy(d01r[0:1, bs], lgb[:])

                # w0 = 1 / (1 + exp((l1+b2[1]) - (l0+b2[0])))
                d01 = smp.tile([1, r], f32, tag="d01")
                nc.vector.tensor_scalar(out=d01[:], in0=d01r[:],
                                        scalar1=b2r[0:1, 1:2],
                                        scalar2=b2r[0:1, 0:1],
                                        op0=OP.add, op1=OP.subtract)
                dps = mlpps.tile([P, nblk], f32, tag="dps")
                for blk in range(nblk):
                    nc.tensor.matmul(dps[:, blk:blk + 1],
                                     lhsT=d01[0:1, blk * P:(blk + 1) * P],
                                     rhs=ones11[:], start=True, stop=True)
                eD = smp.tile([P, nblk], f32, tag="ed")
                nc.scalar.activation(eD[:], dps[:], AF.Exp)
                den = smp.tile([P, nblk], f32, tag="den")
                nc.vector.tensor_scalar(out=den[:], in0=eD[:], scalar1=1.0,
                                        scalar2=None, op0=OP.add)
                w0 = smp.tile([P, nblk], f32, tag="w0")
                nc.vector.reciprocal(w0[:], den[:])
                w1c = smp.tile([P, nblk], f32, tag="w1c")
                nc.vector.tensor_scalar(out=w1c[:], in0=w0[:], scalar1=-1.0,
                                        scalar2=1.0, op0=OP.mult, op1=OP.add)

                wout = smp.tile([P, nblk, 2], f32, tag="wout")
                nc.vector.tensor_copy(wout[:, :, 0:1],
                                      w0[:].rearrange("p (b u) -> p b u", u=1))
                nc.vector.tensor_copy(wout[:, :, 1:2],
                                      w1c[:].rearrange("p (b u) -> p b u", u=1))
                nc.sync.dma_start(
                    out=wout_e[:].rearrange("(b p) c -> p b c", p=P),
                    in_=wout[:])

                # ---- loss tail ----
                wp = smp.tile([P, nblk], f32, tag="wp")
                t4 = smp.tile([P, nblk], f32, tag="t4")
                nc.vector.tensor_tensor(out=wp[:], in0=w0[:], in1=posF[:],
                                        op=OP.mult)
                nc.vector.tensor_tensor(out=t4[:], in0=w1c[:], in1=posP[:],
                                        op=OP.mult)
                nc.vector.tensor_tensor(out=wp[:], in0=wp[:], in1=t4[:],
                                        op=OP.add)
                dn = smp.tile([P, nblk], f32, tag="dn")
                nc.vector.tensor_tensor(out=dn[:], in0=w0[:], in1=totF[:],
                                        op=OP.mult)
                nc.vector.tensor_tensor(out=t4[:], in0=w1c[:], in1=totP[:],
                                        op=OP.mult)
                nc.vector.tensor_tensor(out=dn[:], in0=dn[:], in1=t4[:],
                                        op=OP.add)
                nei = smp.tile([P, nblk], f32, tag="nei")
                nc.vector.tensor_tensor(out=nei[:], in0=degF[:], in1=degP[:],
                                        op=OP.add)
                nc.vector.tensor_scalar(out=nei[:], in0=nei[:], scalar1=1.0,
                                        scalar2=None, op0=OP.max)
                nc.vector.tensor_tensor(out=dn[:], in0=dn[:], in1=nei[:],
                                        op=OP.mult)
                rec = smp.tile([P, nblk], f32, tag="rec")
                nc.vector.reciprocal(rec[:], dn[:])
                ratio = smp.tile([P, nblk], f32, tag="ratio")
                nc.vector.tensor_tensor(out=ratio[:], in0=wp[:], in1=rec[:],
                                        op=OP.mult)
                nc.vector.tensor_scalar(out=ratio[:], in0=ratio[:],
                                        scalar1=1e-10, scalar2=None, op0=OP.max)
                lnr = smp.tile([P, nblk], f32, tag="lnr")
                lsum = smp.tile([P, 1], f32, tag="lsum")
                nc.scalar.activation(lnr[:], ratio[:], AF.Ln, accum_out=lsum[:])
                lp = mlpps.tile([1, 1], f32, tag="dps")
                nc.tensor.matmul(lp[:], lhsT=lsum[:], rhs=onescol[:],
                                 start=True, stop=True)
                lps = smp.tile([1, 1], f32, tag="lps")
                nc.vector.tensor_scalar(out=lps[:], in0=lp[:],
                                        scalar1=-1.0 / n, scalar2=None,
                                        op0=OP.mult)

                ib = dramp.tile([1, 1], f32, tag="ib")
                ob = dramp.tile([1, 1], f32, tag="ob")
                nc.gpsimd.dma_start(out=ib[:], in_=lps[:])
                nc.gpsimd.collective_compute(
                    "AllReduce", OP.add,
                    replica_groups=[list(range(NCORES))],
                    ins=[ib.opt()], outs=[ob.opt()])
                nc.gpsimd.dma_start(out=loss_e[:], in_=ob[:])

    _dedupe_ldweights(nc)
    _split_multi_waits(nc)
    return nc


def _prep_inputs(embF, embM, embP, FM_adj, FP_adj, W1, b1, W2, b2, n=N):
    """Host-side layout-only prep: dtype casts, slices, transposes."""
    r = n // NCORES
    f16 = np.float16
    f32 = np.float32
    aF = FM_adj.astype(f16)
    aP = FP_adj.astype(f16)
    def swz_emb(e):
        # [n, D] -> [128, (n/128)*D]: row p holds rows {a*128+p} concatenated
        x = e.astype(f16).reshape(n // P, P, D).transpose(1, 0, 2)
        return np.ascontiguousarray(x.reshape(P, (n // P) * D))

    def swz_adjT(aT):
        # [n, r] -> [n/4, 4r]: row (jb*128+p) holds chunks c=0..3 of superblock
        x = aT.reshape(n // 512, 4, P, r).transpose(0, 2, 1, 3)
        return np.ascontiguousarray(x.reshape(n // 4, 4 * r))

    shared = {
        "embMn": swz_emb(embM),
        "embPn": swz_emb(embP),
        "embMT": np.ascontiguousarray(embM.T.astype(f16)),
        "embPT": np.ascontiguousarray(embP.T.astype(f16)),
        "w1m": np.ascontiguousarray(W1[:D].astype(f16)),
        "w1p": np.ascontiguousarray(W1[D:].astype(f16)),
        "w2": np.ascontiguousarray(W2.astype(f16)),
        "b1r": np.ascontiguousarray(b1.reshape(1, D).astype(f32)),
        "b2r": np.ascontiguousarray(b2.reshape(1, 2).astype(f32)),
        "ones16": np.ones((P, 1), f16),
        "onescol": np.ones((P, 1), f32),
        "ones11": np.ones((1, 1), f32),
        "ident": np.eye(P, dtype=f16),
    }
    in_maps = []
    for c in range(NCORES):
        I = slice(c * r, (c + 1) * r)
        m = dict(shared)
        m["adjF"] = np.ascontiguousarray(aF[I])
        m["adjP"] = np.ascontiguousarray(aP[I])
        m["adjFT"] = swz_adjT(np.ascontiguousarray(aF[I].T))
        m["adjPT"] = swz_adjT(np.ascontiguousarray(aP[I].T))
        m["embFT"] = np.ascontiguousarray(embF[I].T.astype(f16))
        in_maps.append(m)
    return in_maps


def run(inputs, n=N, trace=False, trace_kwargs=None):
    from concourse.bass_utils import run_bass_kernel_spmd

    key = n
    if key not in _CACHE:
        _CACHE[key] = build(n)
    nc = _CACHE[key]
    in_maps = _prep_inputs(n=n, **inputs)
    res = run_bass_kernel_spmd(nc, in_maps, list(range(NCORES)),
                               trace=trace, **(trace_kwargs or {}))
    r = n // NCORES
    weights = np.concatenate(
        [res.results[c]["w_out"] for c in range(NCORES)], axis=0)
    loss = np.float32(res.results[0]["loss_out"][0, 0])
    return loss, weights.astype(np.float32), res


def kernel(embF, embM, embP, FM_adj, FP_adj, W1, b1, W2, b2):
    loss, weights, _ = run(dict(
        embF=np.asarray(embF), embM=np.asarray(embM), embP=np.asarray(embP),
        FM_adj=np.asarray(FM_adj), FP_adj=np.asarray(FP_adj),
        W1=np.asarray(W1), b1=np.asarray(b1), W2=np.asarray(W2),
        b2=np.asarray(b2)))
    return loss, weights


# revision 34
# speedup vs baseline: 1.4437x; 1.0046x over previous
"""Trainium2 Bass kernel for AdaptiveWeightedFConLoss (8 NeuronCores, SPMD).

Strategy (sharding_hint): anchor rows are sharded across the 8 cores; each
core owns 768 rows of embF / FM_adj / FP_adj; embM / embP are replicated.
Host-side work is layout-only (dtype cast to fp16, slicing, transposition);
all arithmetic happens on the device.

Per-core device pipeline:
  - l2-norm scales computed on-device (squares -> ones-matmul row sums ->
    exp(-0.5*ln(s)) to stay in one ACT table set).
  - Aggregation (FM_adj @ embM) as PE matmuls over host-transposed adjacency
    tiles; result kept transposed [d, i] which feeds the MLP matmuls directly.
  - Similarity exp(cos/tau): PE matmul -> ScalarE exp with accum_out (row
    totals ride for free) -> fused DVE tensor_tensor_reduce for the masked
    positive sums; degree row-sums via DVE tensor_scalar accum.
  - 2-class softmax via exp+reciprocal (no sigmoid table), loss tail reduced
    on-chip, AllReduce of the scalar partial across the 8 cores.
"""

import numpy as np

N = 6144
D = 128
TAU = 0.1
NCORES = 8
P = 128
R = N // NCORES          # 768 anchor rows per core
GRP = 1024               # sim column group (2 psum banks)

_CACHE = {}


def _patch_tile_drain():
    """walrus in this container only allows one semaphore wait per CTRL
    (Drain) instruction; split the TileContext exit-drain waits across
    single-wait NOPs."""
    from concourse import tile, mybir
    from concourse.tile import ScopedClock

    if getattr(tile.TileContext, "_drain_patched", False):
        return

    def _drain_and_barrier(self, tick_clock, wait_clock):
        nc = self.nc
        drain_inst = nc.sync.drain()
        wait_clock.add_sem_waits(
            drain_inst.ins, ScopedClock({None: tick_clock.global_clock})
        )
        si = drain_inst.ins.sync_info
        if si is not None and si.on_wait is not None and len(si.on_wait) > 1:
            waits = list(si.on_wait)
            del si.on_wait[1:]
            for w in waits[1:]:
                n = nc.sync.nop(nofuse=True)
                n.ins.sync_info = mybir.SyncInfo(on_wait=[w], on_update=[])
        nc.all_engine_barrier()
        popped = nc._tile_sem_poison_stack.pop()
        assert popped is self._sem_poison
        nc.clear_and_free_semaphores(list(self.sems.allocated().values()))
        nc.all_engine_barrier()

    tile.TileContext._drain_and_barrier = _drain_and_barrier
    tile.TileContext._drain_patched = True


def _dedupe_ldweights(nc):
    """Tile legalization inserts an InstLdweights before every matmul, even
    when the PE array already holds those weights; drop the redundant ones so
    same-weight matmuls issue back-to-back."""
    removed = 0
    for fn in nc.m.functions:
        for bb in fn.blocks:
            out = []
            last_key = [None]
            pending = []
            for ins in bb.instructions:
                tn = type(ins).__name__
                if tn == "InstLdweights":
                    ap = ins.ins[0]
                    try:
                        key = (ap.memref, ap.offset, str(ap.ap), str(ap.dtype),
                               ins.is_transpose, ins.perf_mode)
                    except AttributeError:
                        key = object()
                    si = ins.sync_info
                    has_upd = bool(si and si.on_update)
                    if key == last_key[0] and not has_upd:
                        removed += 1
                        if si and si.on_wait:
                            pending.extend(si.on_wait)
                        continue
                    last_key[0] = key
                elif tn == "InstMatmult":
                    if pending:
                        from concourse import mybir
                        si = ins.sync_info
                        if si is None:
                            ins.sync_info = mybir.SyncInfo(
                                on_wait=list(pending), on_update=[])
                        else:
                            si.on_wait = list(si.on_wait or []) + pending
                        pending = []
                out.append(ins)
            assert not pending
            bb.instructions[:] = out
    return removed


def _merge_mm_incs(nc):
    """Every matmul gets a completion-semaphore update from Tile, which
    forces isolated-instruction latency ((398+N)/2.4 instead of N/2.4).
    Within a run of consecutive PE matmuls/ldweights with no intervening
    semaphore waits, defer all updates onto the last matmul of the run —
    matmuls complete in order, so this is equivalent and lets earlier
    ones pipeline."""
    from concourse import mybir

    for fn in nc.m.functions:
        for bb in fn.blocks:
            pe = [ins for ins in bb.instructions
                  if ins.engine == mybir.EngineType.PE]
            carry = []
            prev_mm = None
            for ins in pe:
                tn = type(ins).__name__
                si = ins.sync_info
                has_wait = bool(si and si.on_wait)
                if tn == "InstMatmult" and not has_wait and prev_mm is not None:
                    psi = prev_mm.sync_info
                    if psi is not None and psi.on_update:
                        carry.extend(psi.on_update)
                        del psi.on_update[:]
                    prev_mm = ins
                elif tn == "InstMatmult":
                    # wait present: flush carry onto the previous matmul
                    if carry and prev_mm is not None:
                        psi = prev_mm.sync_info
                        if psi is None:
                            prev_mm.sync_info = mybir.SyncInfo(
                                on_wait=[], on_update=list(carry))
                        else:
                            psi.on_update = list(psi.on_update or []) + carry
                        carry = []
                    prev_mm = ins
                elif tn in ("InstLdweights", "InstNoOp") and not has_wait:
                    continue
                else:
                    # anything else (or waiting inst) ends the run
                    if carry and prev_mm is not None:
                        psi = prev_mm.sync_info
                        if psi is None:
                            prev_mm.sync_info = mybir.SyncInfo(
                                on_wait=[], on_update=list(carry))
                        else:
                            psi.on_update = list(psi.on_update or []) + carry
                        carry = []
                    prev_mm = None
            if carry and prev_mm is not None:
                psi = prev_mm.sync_info
                if psi is None:
                    prev_mm.sync_info = mybir.SyncInfo(
                        on_wait=[], on_update=list(carry))
                else:
                    psi.on_update = list(psi.on_update or []) + carry
            # fuse same-semaphore updates (HW allows one update per inst)
            for ins in pe:
                si = ins.sync_info
                if si is None or not si.on_update or len(si.on_update) < 2:
                    continue
                fused = {}
                order = []
                for u in si.on_update:
                    k = (u.sync_type, u.id, u.update_mode)
                    if k in fused:
                        fused[k].update_value += u.update_value
                    else:
                        fused[k] = u
                        order.append(k)
                si.on_update[:] = [fused[k] for k in order]


def _split_multi_waits(nc, limit=1):
    """This container's walrus allows only one semaphore wait per
    instruction; move extra waits onto same-engine NOPs inserted before."""
    from concourse import mybir

    cnt = 0
    for fn in nc.m.functions:
        for bb in fn.blocks:
            out = []
            for ins in bb.instructions:
                si = ins.sync_info
                if si is not None and si.on_wait and len(si.on_wait) > limit:
                    waits = list(si.on_wait)
                    del si.on_wait[limit:]
                    for w in waits[limit:]:
                        cnt += 1
                        nop = mybir.InstNoOp(
                            name=f"I-wsplit-{cnt}", ins=[], outs=[])
                        nop.engine = ins.engine
                        nop.sync_info = mybir.SyncInfo(
                            on_wait=[w], on_update=[])
                        out.append(nop)
                out.append(ins)
            bb.instructions[:] = out


def _patch_ldw_opt():
    """Enable walrus's LDWEIGHTS dedup pass (hardcoded off in bass_utils):
    consecutive same-weight matmuls then pipeline back-to-back."""
    from concourse import bass_utils

    if getattr(bass_utils, "_ldw_opt_patched", False):
        return
    orig = bass_utils.run_command

    def run_command(cmd, *a, **kw):
        if isinstance(cmd, list):
            cmd = ["--enable-ldw-opt=true" if c == "--enable-ldw-opt=false"
                   else c for c in cmd]
        return orig(cmd, *a, **kw)

    bass_utils.run_command = run_command
    bass_utils._ldw_opt_patched = True


def build(n=N):
    """Build the SPMD Bass program for one core (all cores identical)."""
    from concourse import bass, tile, mybir

    _patch_tile_drain()

    f16 = mybir.dt.float16
    f32 = mybir.dt.float32
    AF = mybir.ActivationFunctionType
    OP = mybir.AluOpType
    AX = mybir.AxisListType.X

    r = n // NCORES
    nblk = r // P
    njc = n // P
    njb = njc // 4
    grp = GRP if n % GRP == 0 else 512
    ngrp = n // grp
    nsub = grp // 512
    inv_tau = 1.0 / TAU

    nc = bass.Bass()
    dp = nc.declare_dram_parameter
    adjF_e = dp("adjF", [r, n], f16, isOutput=False)
    adjP_e = dp("adjP", [r, n], f16, isOutput=False)
    adjFT_e = dp("adjFT", [n // 4, 4 * r], f16, isOutput=False)
    adjPT_e = dp("adjPT", [n // 4, 4 * r], f16, isOutput=False)
    embMn_e = dp("embMn", [P, (n // P) * D], f16, isOutput=False)
    embPn_e = dp("embPn", [P, (n // P) * D], f16, isOutput=False)
    embMT_e = dp("embMT", [P, n], f16, isOutput=False)
    embPT_e = dp("embPT", [P, n], f16, isOutput=False)
    embFT_e = dp("embFT", [P, r], f16, isOutput=False)
    w1m_e = dp("w1m", [P, P], f16, isOutput=False)
    w1p_e = dp("w1p", [P, P], f16, isOutput=False)
    w2_e = dp("w2", [P, 2], f16, isOutput=False)
    b1r_e = dp("b1r", [1, P], f32, isOutput=False)
    b2r_e = dp("b2r", [1, 2], f32, isOutput=False)
    ones16_e = dp("ones16", [P, 1], f16, isOutput=False)
    onescol_e = dp("onescol", [P, 1], f32, isOutput=False)
    ones11_e = dp("ones11", [1, 1], f32, isOutput=False)
    ident_e = dp("ident", [P, P], f16, isOutput=False)
    wout_e = dp("w_out", [r, 2], f32, isOutput=True)
    loss_e = dp("loss_out", [1, 1], f32, isOutput=True)
    dbg_e = dp("dbg", [P, 4 * (r // P)], f32, isOutput=True)

    with tile.TileContext(nc) as tc:
        with (
            tc.tile_pool(name="const", bufs=1) as cp,
            tc.tile_pool(name="resident", bufs=1) as rp,
            tc.tile_pool(name="stage", bufs=1) as sp,
            tc.tile_pool(name="adjt", bufs=3) as adjtp,
            tc.tile_pool(name="adjb", bufs=4) as adjbp,
            tc.tile_pool(name="sim", bufs=3) as simp,
            tc.tile_pool(name="msk", bufs=2) as mskp,
            tc.tile_pool(name="small", bufs=1) as smp,
            tc.tile_pool(name="dram", bufs=1, space="DRAM") as dramp,
        ):
            # ---- consts ----
            ones16 = cp.tile([P, 1], f16, tag="c0")
            nc.sync.dma_start(out=ones16[:], in_=ones16_e[:])
            onescol = cp.tile([P, 1], f32, tag="c1")
            nc.sync.dma_start(out=onescol[:], in_=onescol_e[:])
            ones11 = cp.tile([1, 1], f32, tag="c2")
            nc.sync.dma_start(out=ones11[:], in_=ones11_e[:])
            ident = cp.tile([P, P], f16, tag="c3")
            nc.sync.dma_start(out=ident[:], in_=ident_e[:])
            w1m = cp.tile([P, P], f16, tag="c4")
            nc.sync.dma_start(out=w1m[:], in_=w1m_e[:])
            w1p = cp.tile([P, P], f16, tag="c5")
            nc.sync.dma_start(out=w1p[:], in_=w1p_e[:])
            w2 = cp.tile([P, 2], f16, tag="c6")
            nc.sync.dma_start(out=w2[:], in_=w2_e[:])
            b2r = cp.tile([1, 2], f32, tag="c8")
            nc.sync.dma_start(out=b2r[:], in_=b2r_e[:])
            b1bc = cp.tile([P, P], f32, tag="c9")
            nc.sync.dma_start(out=b1bc[:], in_=b1r_e[0:1, :].broadcast_to([P, P]))

            # warm-up AllReduce: establishes the collective channel and
            # absorbs cross-core launch skew while compute streams
            wu = smp.tile([1, 1], f32, tag="wu")
            nc.vector.memset(wu[:], 0.0)
            wub_i = dramp.tile([1, 1], f32, tag="wub_i")
            wub_o = dramp.tile([1, 1], f32, tag="wub_o")
            nc.gpsimd.dma_start(out=wub_i[:], in_=wu[:])
            nc.gpsimd.collective_compute(
                "AllReduce", OP.add,
                replica_groups=[list(range(NCORES))],
                ins=[wub_i.opt()], outs=[wub_o.opt()])

            # ---- normalization scales via ones-matmul row sums ----
            # rn_row = exp(-0.5*ln(colsum(x^2)))  (stays in the exp/ln table)
            def norm_scale(embT, width, rowps, name):
                sq = sp.tile([P, width], f16, tag="sqx", name="sq")
                nc.vector.tensor_tensor(out=sq[:], in0=embT[:], in1=embT[:],
                                        op=OP.mult)
                rnx = sp.tile([1, width], f16, tag="rnx", name="rnx")
                for s in range(0, width, 512):
                    w = min(512, width - s)
                    nrm = rowps.tile([1, 512], f32, tag="nrm", name="nrm")
                    nc.tensor.matmul(nrm[:, 0:w], lhsT=ones16[:],
                                     rhs=sq[:, s:s + w], start=True, stop=True)
                    lnp = sp.tile([1, 512], f32, tag="lnp", name="lnp")
                    nc.scalar.activation(lnp[0:1, 0:w], nrm[:, 0:w], AF.Ln)
                    nc.scalar.activation(rnx[0:1, s:s + w], lnp[0:1, 0:w],
                                         AF.Exp, scale=-0.5)
                rnd = dramp.tile([1, width], f16, tag="rnd_" + name)
                nc.sync.dma_start(out=rnd[:], in_=rnx[:])
                rnb = sp.tile([P, width], f16, tag="rnb", name="rnb")
                nc.sync.dma_start(out=rnb[:],
                                  in_=rnd[0:1, :].broadcast_to([P, width]))
                out = rp.tile([P, width], f16, tag="nt_" + name)
                nc.vector.tensor_tensor(out=out[:], in0=embT[:], in1=rnb[:],
                                        op=OP.mult)
                return out

            with tc.tile_pool(name="rowps", bufs=2, space="PSUM") as rowps:
                embFT = sp.tile([P, r], f16, tag="eft")
                nc.sync.dma_start(out=embFT[:], in_=embFT_e[:])
                FnT = norm_scale(embFT, r, rowps, "f")
                embMT = sp.tile([P, n], f16, tag="ext", name="embXT")
                nc.sync.dma_start(out=embMT[:], in_=embMT_e[:])
                MnT = norm_scale(embMT, n, rowps, "m")
                embPT = sp.tile([P, n], f16, tag="ext", name="embXT2")
                nc.sync.dma_start(out=embPT[:], in_=embPT_e[:])
                PnT = norm_scale(embPT, n, rowps, "p")

            # ---- interleaved schedule: aggregation batches woven between
            # sim/mask units so PE, ACT, DVE and both DMA streams overlap ----
            rawT = []
            degrows = []
            rw1 = min(512, r)

            with (
                tc.tile_pool(name="aggps", bufs=1, space="PSUM") as aggps,
                tc.tile_pool(name="simps", bufs=2, space="PSUM") as simps,
            ):
                posF_p = smp.tile([P, nblk * ngrp], f32, tag="posf")
                posP_p = smp.tile([P, nblk * ngrp], f32, tag="posp")
                totF_p = smp.tile([P, nblk * ngrp], f32, tag="totf")
                totP_p = smp.tile([P, nblk * ngrp], f32, tag="totp")
                posF = smp.tile([P, nblk], f32, tag="posfr")
                posP = smp.tile([P, nblk], f32, tag="pospr")
                totF = smp.tile([P, nblk], f32, tag="totfr")
                totP = smp.tile([P, nblk], f32, tag="totpr")

                # sim work: per block -> load adj rows, then per (rel, grp)
                sim_state = {"blk": -1, "adjF": None, "adjP": None}
                sim_work = [(blk, ri, g) for blk in range(nblk)
                            for ri in range(2) for g in range(ngrp)]
                sim_pos = [0]

                def emit_block_load(blk):
                    adjFb = adjbp.tile([P, n], f16, tag="adjb", name="adjFb")
                    nc.sync.dma_start(out=adjFb[:],
                                      in_=adjF_e[blk * P:(blk + 1) * P, :])
                    adjPb = adjbp.tile([P, n], f16, tag="adjb", name="adjPb")
                    nc.sync.dma_start(out=adjPb[:],
                                      in_=adjP_e[blk * P:(blk + 1) * P, :])
                    sim_state["blk"] = blk
                    sim_state["adjF"] = adjFb
                    sim_state["adjP"] = adjPb

                def emit_block_reduce(blk):
                    for dst, srcp in ((posF, posF_p), (posP, posP_p),
                                      (totF, totF_p), (totP, totP_p)):
                        nc.vector.tensor_reduce(
                            out=dst[:, blk:blk + 1].rearrange(
                                "p (b u) -> p b u", u=1),
                            in_=srcp[:, blk * ngrp:(blk + 1) * ngrp].rearrange(
                                "p (b g) -> p b g", g=ngrp),
                            axis=AX, op=OP.add)

                def emit_sim(count):
                    for _ in range(count):
                        if sim_pos[0] >= len(sim_work):
                            return
                        blk, ri, g = sim_work[sim_pos[0]]
                        sim_pos[0] += 1
                        if sim_state["blk"] != blk:
                            emit_block_load(blk)
                        adjb = sim_state["adjF"] if ri == 0 else sim_state["adjP"]
                        XT = MnT if ri == 0 else PnT
                        pos_p = posF_p if ri == 0 else posP_p
                        tot_p = totF_p if ri == 0 else totP_p
                        sps = simps.tile([P, grp], f32, tag="simps",
                                         name="sps")
                        for s in range(nsub):
                            c0 = g * grp + s * 512
                            nc.tensor.matmul(
                                sps[:, s * 512:(s + 1) * 512],
                                lhsT=FnT[:, blk * P:(blk + 1) * P],
                                rhs=XT[:, c0:c0 + 512],
                                start=True, stop=True)
                        sim = simp.tile([P, grp], f16, tag="sim", name="sim")
                        idx = blk * ngrp + g
                        nc.scalar.activation(
                            sim[:], sps[:], AF.Exp, scale=inv_tau,
                            accum_out=tot_p[:, idx:idx + 1])
                        msk = mskp.tile([P, grp], f16, tag="msk", name="msk")
                        nc.vector.scalar_tensor_tensor(
                            out=msk[:], in0=sim[:], scalar=1.0,
                            in1=adjb[:, g * grp:(g + 1) * grp],
                            op0=OP.mult, op1=OP.mult,
                            accum_out=pos_p[:, idx:idx + 1])


                # ~3 sim units per agg batch balances PE/ACT/DVE streams
                per = 3

                for name, embXn_e, adjXT_e in (("m", embMn_e, adjFT_e),
                                               ("p", embPn_e, adjPT_e)):
                    embXn = rp.tile([P, njc, D], f16, tag="exn_" + name,
                                    name="exn")
                    nc.sync.dma_start(
                        out=embXn[:],
                        in_=embXn_e[:].rearrange("p (a d) -> p a d", d=D))
                    raw1 = aggps.tile([P, rw1], f32, tag="agg1", name="raw1")
                    raw2 = (aggps.tile([P, r - 512], f32, tag="agg2",
                                       name="raw2") if r > 512 else None)
                    degrow = aggps.tile([1, r], f32, tag="degrow",
                                        name="degrow")
                    for jb in range(njb):
                        bt = adjtp.tile([P, 4, r], f16, tag="adjt", name="bt")
                        nc.sync.dma_start(
                            out=bt[:],
                            in_=adjXT_e[jb * P:(jb + 1) * P, :].rearrange(
                                "p (c i) -> p c i", c=4))
                        for c in range(4):
                            jc = jb * 4 + c
                            st = jc == 0
                            en = jc == njc - 1
                            nc.tensor.matmul(degrow[:, 0:rw1], lhsT=ones16[:],
                                             rhs=bt[:, c, 0:rw1],
                                             start=st, stop=en)
                            if r > 512:
                                nc.tensor.matmul(degrow[:, 512:r],
                                                 lhsT=ones16[:],
                                                 rhs=bt[:, c, 512:r],
                                                 start=st, stop=en)
                        for c in range(4):
                            jc = jb * 4 + c
                            st = jc == 0
                            en = jc == njc - 1
                            nc.tensor.matmul(raw1[:], lhsT=embXn[:, jc, :],
                                             rhs=bt[:, c, 0:rw1],
                                             start=st, stop=en)
                            if raw2 is not None:
                                nc.tensor.matmul(raw2[:], lhsT=embXn[:, jc, :],
                                                 rhs=bt[:, c, 512:r],
                                                 start=st, stop=en)
                        emit_sim(per)
                    rt = rp.tile([P, r], f16, tag="rawt_" + name)
                    nc.vector.tensor_copy(rt[:, 0:rw1], raw1[:])
                    if raw2 is not None:
                        nc.vector.tensor_copy(rt[:, 512:r], raw2[:])
                    dr = sp.tile([1, r], f32, tag="degrow_sb_" + name,
                                 name="degrow_sb")
                    nc.vector.tensor_copy(dr[:], degrow[:])
                    rawT.append(rt)
                    degrows.append(dr)
                emit_sim(len(sim_work))
                for dst_, src_ in ((posF, posF_p), (posP, posP_p),
                                   (totF, totF_p), (totP, totP_p)):
                    nc.vector.tensor_reduce(
                        out=dst_[:].rearrange("p (b u) -> p b u", u=1),
                        in_=src_[:].rearrange("p (b g) -> p b g", g=ngrp),
                        axis=AX, op=OP.add)
            rawTM, rawTP = rawT

            with tc.tile_pool(name="mlpps", bufs=1, space="PSUM") as mlpps:
                # deg rows [1, r] -> column layout [P, nblk] via tiny matmuls
                degF = smp.tile([P, nblk], f32, tag="degf")
                degP = smp.tile([P, nblk], f32, tag="degp")
                for dst, drow in ((degF, degrows[0]), (degP, degrows[1])):
                    dcp = mlpps.tile([P, nblk], f32, tag="dps", name="dcp")
                    for blk in range(nblk):
                        nc.tensor.matmul(dcp[:, blk:blk + 1],
                                         lhsT=drow[0:1, blk * P:(blk + 1) * P],
                                         rhs=ones11[:], start=True, stop=True)
                    nc.vector.tensor_copy(dst[:], dcp[:])

                # ---- MLP -> weights ----
                degFc = smp.tile([P, nblk], f32, tag="degfc")
                nc.vector.tensor_scalar(out=degFc[:], in0=degF[:], scalar1=1.0,
                                        scalar2=None, op0=OP.max)
                degPc = smp.tile([P, nblk], f32, tag="degpc")
                nc.vector.tensor_scalar(out=degPc[:], in0=degP[:], scalar1=1.0,
                                        scalar2=None, op0=OP.max)
                rdF = smp.tile([P, nblk], f32, tag="rdf")
                nc.vector.reciprocal(rdF[:], degFc[:])
                rdP = smp.tile([P, nblk], f32, tag="rdp")
                nc.vector.reciprocal(rdP[:], degPc[:])

                w2d = smp.tile([P, 1], f16, tag="w2d")
                nc.vector.tensor_tensor(out=w2d[:], in0=w2[:, 1:2],
                                        in1=w2[:, 0:1], op=OP.subtract)
                d01r = smp.tile([1, r], f32, tag="d01r")
                for blk in range(nblk):
                    bs = slice(blk * P, (blk + 1) * P)
                    u = mlpps.tile([P, P], f32, tag="mlpuv")
                    nc.tensor.matmul(u[:], lhsT=rawTM[:, bs], rhs=w1m[:],
                                     start=True, stop=True)
                    t1 = smp.tile([P, P], f32, tag="t1")
                    nc.vector.tensor_scalar(out=t1[:], in0=u[:],
                                            scalar1=rdF[:, blk:blk + 1],
                                            scalar2=None, op0=OP.mult)
                    v = mlpps.tile([P, P], f32, tag="mlpuv")
                    nc.tensor.matmul(v[:], lhsT=rawTP[:, bs], rhs=w1p[:],
                                     start=True, stop=True)
                    t2 = smp.tile([P, P], f32, tag="t2")
                    nc.vector.scalar_tensor_tensor(
                        out=t2[:], in0=v[:], scalar=rdP[:, blk:blk + 1],
                        in1=t1[:], op0=OP.mult, op1=OP.add)
                    t3 = smp.tile([P, P], f32, tag="t3")
                    nc.vector.tensor_tensor(out=t3[:], in0=t2[:], in1=b1bc[:],
                                            op=OP.add)
                    h16 = smp.tile([P, P], f16, tag="h16")
                    nc.vector.tensor_scalar(out=h16[:], in0=t3[:], scalar1=0.0,
                                            scalar2=None, op0=OP.max)
                    hT = mlpps.tile([P, P], f16, tag="ht")
                    nc.tensor.transpose(hT[:], h16[:], ident[:])
                    hT16 = smp.tile([P, P], f16, tag="ht16")
                    nc.vector.tensor_copy(hT16[:], hT[:])
                    lgb = mlpps.tile([1, P], f32, tag="lgb")
                    nc.tensor.matmul(lgb[:], lhsT=w2d[:], rhs=hT16[:],
                                     start=True, stop=True)
                    nc.vector.tensor_copy(d01r[0:1, bs], lgb[:])

                # w0 = 1 / (1 + exp((l1+b2[1]) - (l0+b2[0])))
                d01 = smp.tile([1, r], f32, tag="d01")
                nc.vector.tensor_scalar(out=d01[:], in0=d01r[:],
                                        scalar1=b2r[0:1, 1:2],
                                        scalar2=b2r[0:1, 0:1],
                                        op0=OP.add, op1=OP.subtract)
                dps = mlpps.tile([P, nblk], f32, tag="dps")
                for blk in range(nblk):
                    nc.tensor.matmul(dps[:, blk:blk + 1],
                                     lhsT=d01[0:1, blk * P:(blk + 1) * P],
                                     rhs=ones11[:], start=True, stop=True)
                eD = smp.tile([P, nblk], f32, tag="ed")
                nc.scalar.activation(eD[:], dps[:], AF.Exp)
                den = smp.tile([P, nblk], f32, tag="den")
                nc.vector.tensor_scalar(out=den[:], in0=eD[:], scalar1=1.0,
                                        scalar2=None, op0=OP.add)
                w0 = smp.tile([P, nblk], f32, tag="w0")
                nc.vector.reciprocal(w0[:], den[:])
                w1c = smp.tile([P, nblk], f32, tag="w1c")
                nc.vector.tensor_scalar(out=w1c[:], in0=w0[:], scalar1=-1.0,
                                        scalar2=1.0, op0=OP.mult, op1=OP.add)

                wout = smp.tile([P, nblk, 2], f32, tag="wout")
                nc.vector.tensor_copy(wout[:, :, 0:1],
                                      w0[:].rearrange("p (b u) -> p b u", u=1))
                nc.vector.tensor_copy(wout[:, :, 1:2],
                                      w1c[:].rearrange("p (b u) -> p b u", u=1))
                nc.sync.dma_start(
                    out=wout_e[:].rearrange("(b p) c -> p b c", p=P),
                    in_=wout[:])

                # ---- loss tail ----
                wp = smp.tile([P, nblk], f32, tag="wp")
                t4 = smp.tile([P, nblk], f32, tag="t4")
                nc.vector.tensor_tensor(out=wp[:], in0=w0[:], in1=posF[:],
                                        op=OP.mult)
                nc.vector.tensor_tensor(out=t4[:], in0=w1c[:], in1=posP[:],
                                        op=OP.mult)
                nc.vector.tensor_tensor(out=wp[:], in0=wp[:], in1=t4[:],
                                        op=OP.add)
                dn = smp.tile([P, nblk], f32, tag="dn")
                nc.vector.tensor_tensor(out=dn[:], in0=w0[:], in1=totF[:],
                                        op=OP.mult)
                nc.vector.tensor_tensor(out=t4[:], in0=w1c[:], in1=totP[:],
                                        op=OP.mult)
                nc.vector.tensor_tensor(out=dn[:], in0=dn[:], in1=t4[:],
                                        op=OP.add)
                nei = smp.tile([P, nblk], f32, tag="nei")
                nc.vector.tensor_tensor(out=nei[:], in0=degF[:], in1=degP[:],
                                        op=OP.add)
                nc.vector.tensor_scalar(out=nei[:], in0=nei[:], scalar1=1.0,
                                        scalar2=None, op0=OP.max)
                nc.vector.tensor_tensor(out=dn[:], in0=dn[:], in1=nei[:],
                                        op=OP.mult)
                rec = smp.tile([P, nblk], f32, tag="rec")
                nc.vector.reciprocal(rec[:], dn[:])
                ratio = smp.tile([P, nblk], f32, tag="ratio")
                nc.vector.tensor_tensor(out=ratio[:], in0=wp[:], in1=rec[:],
                                        op=OP.mult)
                nc.vector.tensor_scalar(out=ratio[:], in0=ratio[:],
                                        scalar1=1e-10, scalar2=None, op0=OP.max)
                lnr = smp.tile([P, nblk], f32, tag="lnr")
                lsum = smp.tile([P, 1], f32, tag="lsum")
                nc.scalar.activation(lnr[:], ratio[:], AF.Ln, accum_out=lsum[:])
                lp = mlpps.tile([1, 1], f32, tag="dps")
                nc.tensor.matmul(lp[:], lhsT=lsum[:], rhs=onescol[:],
                                 start=True, stop=True)
                lps = smp.tile([1, 1], f32, tag="lps")
                nc.vector.tensor_scalar(out=lps[:], in0=lp[:],
                                        scalar1=-1.0 / n, scalar2=None,
                                        op0=OP.mult)

                dbg = smp.tile([P, 4 * nblk], f32, tag="dbg")
                nc.vector.tensor_copy(dbg[:, 0:nblk], posP[:])
                nc.vector.tensor_copy(dbg[:, nblk:2 * nblk], totP[:])
                nc.vector.tensor_copy(dbg[:, 2 * nblk:3 * nblk], w0[:])
                nc.vector.tensor_copy(dbg[:, 3 * nblk:4 * nblk], ratio[:])
                nc.sync.dma_start(out=dbg_e[:], in_=dbg[:])

                ib = dramp.tile([1, 1], f32, tag="ib")
                ob = dramp.tile([1, 1], f32, tag="ob")
                nc.gpsimd.dma_start(out=ib[:], in_=lps[:])
                nc.gpsimd.collective_compute(
                    "AllReduce", OP.add,
                    replica_groups=[list(range(NCORES))],
                    ins=[ib.opt()], outs=[ob.opt()])
                nc.gpsimd.dma_start(out=loss_e[:], in_=ob[:])

    _dedupe_ldweights(nc)
    _split_multi_waits(nc)
    return nc


def _prep_inputs(embF, embM, embP, FM_adj, FP_adj, W1, b1, W2, b2, n=N):
    """Host-side layout-only prep: dtype casts, slices, transposes."""
    r = n // NCORES
    f16 = np.float16
    f32 = np.float32
    aF = FM_adj.astype(f16)
    aP = FP_adj.astype(f16)
    def swz_emb(e):
        # [n, D] -> [128, (n/128)*D]: row p holds rows {a*128+p} concatenated
        x = e.astype(f16).reshape(n // P, P, D).transpose(1, 0, 2)
        return np.ascontiguousarray(x.reshape(P, (n // P) * D))

    def swz_adjT(aT):
        # [n, r] -> [n/4, 4r]: row (jb*128+p) holds chunks c=0..3 of superblock
        x = aT.reshape(n // 512, 4, P, r).transpose(0, 2, 1, 3)
        return np.ascontiguousarray(x.reshape(n // 4, 4 * r))

    shared = {
        "embMn": swz_emb(embM),
        "embPn": swz_emb(embP),
        "embMT": np.ascontiguousarray(embM.T.astype(f16)),
        "embPT": np.ascontiguousarray(embP.T.astype(f16)),
        "w1m": np.ascontiguousarray(W1[:D].astype(f16)),
        "w1p": np.ascontiguousarray(W1[D:].astype(f16)),
        "w2": np.ascontiguousarray(W2.astype(f16)),
        "b1r": np.ascontiguousarray(b1.reshape(1, D).astype(f32)),
        "b2r": np.ascontiguousarray(b2.reshape(1, 2).astype(f32)),
        "ones16": np.ones((P, 1), f16),
        "onescol": np.ones((P, 1), f32),
        "ones11": np.ones((1, 1), f32),
        "ident": np.eye(P, dtype=f16),
    }
    in_maps = []
    for c in range(NCORES):
        I = slice(c * r, (c + 1) * r)
        m = dict(shared)
        m["adjF"] = np.ascontiguousarray(aF[I])
        m["adjP"] = np.ascontiguousarray(aP[I])
        m["adjFT"] = swz_adjT(np.ascontiguousarray(aF[I].T))
        m["adjPT"] = swz_adjT(np.ascontiguousarray(aP[I].T))
        m["embFT"] = np.ascontiguousarray(embF[I].T.astype(f16))
        in_maps.append(m)
    return in_maps


def run(inputs, n=N, trace=False, trace_kwargs=None):
    from concourse.bass_utils import run_bass_kernel_spmd

    key = n
    if key not in _CACHE:
        _CACHE[key] = build(n)
    nc = _CACHE[key]
    in_maps = _prep_inputs(n=n, **inputs)
    res = run_bass_kernel_spmd(nc, in_maps, list(range(NCORES)),
                               trace=trace, **(trace_kwargs or {}))
    r = n // NCORES
    weights = np.concatenate(
        [res.results[c]["w_out"] for c in range(NCORES)], axis=0)
    loss = np.float32(res.results[0]["loss_out"][0, 0])
    return loss, weights.astype(np.float32), res


def kernel(embF, embM, embP, FM_adj, FP_adj, W1, b1, W2, b2):
    loss, weights, _ = run(dict(
        embF=np.asarray(embF), embM=np.asarray(embM), embP=np.asarray(embP),
        FM_adj=np.asarray(FM_adj), FP_adj=np.asarray(FP_adj),
        W1=np.asarray(W1), b1=np.asarray(b1), W2=np.asarray(W2),
        b2=np.asarray(b2)))
    return loss, weights
